# revision 6
# baseline (speedup 1.0000x reference)
"""Causal multi-head self-attention with RoPE on 8 NeuronCores.

Sharding (hardcoded): core c -> batch b = c // 2, head-group hg = c % 2.
Each core:
  - projects its batch's x with column-sharded WQ/WK/WV (8 heads = 512 dims),
  - applies RoPE (host-precomputed cos/sin tables, adjacent-pair swap done
    on-chip with stream_shuffle),
  - runs causal attention for its 8 heads in transposed layout
    (S^T = [k, q]; softmax denominator comes free from a ones-column
    appended to V; normalization is broadcast via a K=1 matmul),
  - applies the row-sharded WO projection -> partial [T, D] output.
Host sums the two partials per batch (the "all-reduce after WO").
"""

import numpy as np
import ml_dtypes

B, T, D, H = 4, 2048, 1024, 16
DK = 64
HLOC = 8          # heads per core
E = HLOC * DK     # 512, local projection width
NCORES = 8
THETA = 10000.0

_BF16 = ml_dtypes.bfloat16

_cache = {}


def _build(t=T, hloc=HLOC, d=D):
    from contextlib import ExitStack

    import concourse.bacc as bacc
    import concourse.bass as bass  # noqa: F401
    import concourse.mybir as mybir
    import concourse.tile as tile

    f32 = mybir.dt.float32
    bf16 = mybir.dt.bfloat16
    Exp = mybir.ActivationFunctionType.Exp

    e = hloc * DK
    npair = hloc // 2       # head-pair tiles in QT/KT/OT
    dsub = d // 128         # contraction subtiles for projections
    tq = t // 512           # 512-wide q chunks
    tk = t // 128           # 128-wide k tiles
    ep = e // 128           # output-partition tiles for Q/K (= npair)
    swap_mask = [i ^ 1 for i in range(32)]

    nc = bacc.Bacc("TRN2", target_bir_lowering=False, debug=False)

    xT = nc.declare_dram_parameter("xT", [d, t], bf16, False).ap()
    wqT = nc.declare_dram_parameter("wqT", [d, e], bf16, False).ap()
    wkT = nc.declare_dram_parameter("wkT", [d, e], bf16, False).ap()
    wvT = nc.declare_dram_parameter("wvT", [d, e], bf16, False).ap()
    woT = nc.declare_dram_parameter("woT", [e, d], bf16, False).ap()
    cosT = nc.declare_dram_parameter("cosT", [128, t], f32, False).ap()
    sinT = nc.declare_dram_parameter("sinT", [128, t], f32, False).ap()
    trim = nc.declare_dram_parameter("trim", [128, 128], bf16, False).ap()
    y = nc.declare_dram_parameter("y", [t, d], f32, True).ap()

    with tile.TileContext(nc) as tc:
        with ExitStack() as ctx:
            const = ctx.enter_context(tc.tile_pool(name="const", bufs=1))
            ptpool = ctx.enter_context(tc.tile_pool(name="ptp", bufs=3))
            normp = ctx.enter_context(tc.tile_pool(name="normp", bufs=2))
            ysbp = ctx.enter_context(tc.tile_pool(name="ysbp", bufs=3))

            wq_sb = const.tile([128, dsub, e], bf16)
            wk_sb = const.tile([128, dsub, e], bf16)
            wv_sb = const.tile([128, dsub, e], bf16)
            wo_sb = const.tile([128, e // 128, d], bf16)
            trim_sb = const.tile([128, 128], bf16)
            ones_sb = const.tile([1, 64], bf16)
            qt_sb = const.tile([128, npair, t], bf16)
            kt_sb = const.tile([128, npair, t], bf16)
            v_sb = const.tile([128, tk, hloc, DK + 1], bf16)
            ot_sb = const.tile([128, npair, t], bf16)

            nc.sync.dma_start(wq_sb, wqT.rearrange("(n p) e -> p n e", p=128))
            nc.sync.dma_start(wk_sb, wkT.rearrange("(n p) e -> p n e", p=128))
            nc.sync.dma_start(wv_sb, wvT.rearrange("(n p) e -> p n e", p=128))
            nc.sync.dma_start(wo_sb, woT.rearrange("(n p) d -> p n d", p=128))
            nc.sync.dma_start(trim_sb, trim)
            nc.vector.memset(ones_sb, 1.0)
            nc.vector.memset(v_sb[:, :, :, DK : DK + 1], 1.0)

            # ---------------- Phase 1: QKV projections + RoPE ----------------
            with ExitStack() as c1:
                p1c = c1.enter_context(tc.tile_pool(name="p1c", bufs=1))
                rope = c1.enter_context(tc.tile_pool(name="rope", bufs=2))
                ppsum = c1.enter_context(
                    tc.tile_pool(name="ppsum", bufs=2, space="PSUM")
                )

                xt_sb = p1c.tile([128, dsub, t], bf16)
                cos_sb = p1c.tile([128, t], f32)
                sin_sb = p1c.tile([128, t], f32)
                nc.sync.dma_start(xt_sb, xT.rearrange("(n p) t -> p n t", p=128))
                nc.sync.dma_start(cos_sb, cosT)
                nc.sync.dma_start(sin_sb, sinT)

                for wsb, dst in ((wq_sb, qt_sb), (wk_sb, kt_sb)):
                    for ie in range(ep):
                        q_ps = ppsum.tile([128, t], f32, tag="proj")
                        for ds_ in range(dsub):
                            for jt in range(tq):
                                nc.tensor.matmul(
                                    q_ps[:, jt * 512 : (jt + 1) * 512],
                                    lhsT=wsb[:, ds_, ie * 128 : (ie + 1) * 128],
                                    rhs=xt_sb[:, ds_, jt * 512 : (jt + 1) * 512],
                                    start=(ds_ == 0),
                                    stop=(ds_ == dsub - 1),
                                )
                        # RoPE: out = cos * q + sinS * pairswap(q)
                        sw = rope.tile([128, t], f32, tag="sw")
                        nc.vector.stream_shuffle(sw, q_ps[:, :], mask=swap_mask)
                        nc.vector.tensor_mul(dst[:, ie, :], q_ps[:, :], cos_sb)
                        nc.vector.tensor_mul(sw, sw, sin_sb)
                        nc.vector.tensor_add(dst[:, ie, :], dst[:, ie, :], sw)

                for it in range(tk):
                    v_ps = ppsum.tile([128, 512], f32, tag="proj")
                    nfree = min(512, e)
                    for ds_ in range(dsub):
                        nc.tensor.matmul(
                            v_ps[:, :nfree],
                            lhsT=xt_sb[:, ds_, it * 128 : (it + 1) * 128],
                            rhs=wv_sb[:, ds_, :nfree],
                            start=(ds_ == 0),
                            stop=(ds_ == dsub - 1),
                        )
                    nc.vector.tensor_copy(
                        v_sb[:, it, :, 0:DK],
                        v_ps[:, :nfree].rearrange("p (h k) -> p h k", h=hloc),
                    )

            # ---------------- Phase 2: attention per head ----------------
            with ExitStack() as c2:
                spsum = c2.enter_context(
                    tc.tile_pool(name="spsum", bufs=1, space="PSUM")
                )
                opsum = c2.enter_context(
                    tc.tile_pool(name="opsum", bufs=1, space="PSUM")
                )
                for h in range(hloc):
                    pr, h2 = h // 2, h % 2
                    po = 64 * h2
                    o_ps = opsum.tile([128, t], f32, tag="o")
                    for i in range(tk):
                        j0, m = i // 4, i % 4
                        s_ps = spsum.tile([128, t], f32, tag="s")
                        for j in range(j0, tq):
                            nc.tensor.matmul(
                                s_ps[:, j * 512 : (j + 1) * 512],
                                lhsT=kt_sb[po : po + 64, pr, i * 128 : (i + 1) * 128],
                                rhs=qt_sb[po : po + 64, pr, j * 512 : (j + 1) * 512],
                                start=True,
                                stop=True,
                            )
                        pt = ptpool.tile([128, t], bf16, tag="pt")
                        nc.scalar.activation(
                            pt[:, j0 * 512 : t],
                            s_ps[:, j0 * 512 : t],
                            Exp,
                            scale=DK ** (-0.5),
                        )
                        lo = j0 * 512 + m * 128
                        if m > 0:
                            nc.gpsimd.memset(pt[:, j0 * 512 : lo], 0.0)
                        nc.vector.tensor_mul(
                            pt[:, lo : lo + 128], pt[:, lo : lo + 128], trim_sb
                        )
                        for j in range(j0, tq):
                            nc.tensor.matmul(
                                o_ps[0:65, j * 512 : (j + 1) * 512],
                                lhsT=v_sb[:, i, h, :],
                                rhs=pt[:, j * 512 : (j + 1) * 512],
                                start=(i == 0),
                                stop=(i == 4 * j + 3),
                            )
                    # normalize: O[dk, q] / denom[q], denom = row 64 of o_ps
                    recip = normp.tile([1, t], bf16, tag="recip")
                    with nc.allow_low_precision(reason="softmax recip in bf16"):
                        nc.vector.reciprocal(recip[0:1, :], o_ps[64:65, 0:t])
                    bc_ps = spsum.tile([128, t], f32, tag="s")
                    for j in range(tq):
                        nc.tensor.matmul(
                            bc_ps[0:64, j * 512 : (j + 1) * 512],
                            lhsT=ones_sb[0:1, 0:64],
                            rhs=recip[0:1, j * 512 : (j + 1) * 512],
                            start=True,
                            stop=True,
                        )
                    bc_sb = normp.tile([64, t], bf16, tag="bc")
                    nc.vector.tensor_copy(bc_sb, bc_ps[0:64, 0:t])
                    nc.vector.tensor_mul(
                        ot_sb[po : po + 64, pr, :], o_ps[0:64, 0:t], bc_sb
                    )

            # ---------------- Phase 3: WO projection ----------------
            with ExitStack() as c3:
                ypsum = c3.enter_context(
                    tc.tile_pool(name="ypsum", bufs=4, space="PSUM")
                )
                nech = d // 512
                for it in range(tk):
                    y_sb = ysbp.tile([128, d], f32, tag="ysb")
                    for ec in range(nech):
                        y_ps = ypsum.tile([128, 512], f32, tag="y")
                        for dp in range(e // 128):
                            nc.tensor.matmul(
                                y_ps,
                                lhsT=ot_sb[:, dp, it * 128 : (it + 1) * 128],
                                rhs=wo_sb[:, dp, ec * 512 : (ec + 1) * 512],
                                start=(dp == 0),
                                stop=(dp == e // 128 - 1),
                            )
                        nc.vector.tensor_copy(y_sb[:, ec * 512 : (ec + 1) * 512], y_ps)
                    nc.sync.dma_start(y[it * 128 : (it + 1) * 128, :], y_sb)

    nc.compile()
    return nc


def _get_nc():
    if "nc" not in _cache:
        _cache["nc"] = _build()
    return _cache["nc"]


def _host_tables(positions):
    """cos/sin RoPE tables laid out for the on-chip [128, T] tiles."""
    pos = np.asarray(positions, np.float32)  # [t]
    inv = 1.0 / THETA ** (
        (2.0 * np.arange(1, DK // 2 + 1, dtype=np.float32) - 2.0) / DK
    )  # [32]
    ang = pos[None, :] * inv[:, None]  # [32, t]
    c32 = np.cos(ang)
    s32 = np.sin(ang)
    rows = np.arange(128)
    dloc = rows % DK
    fidx = dloc // 2
    sign = np.where(dloc % 2 == 0, -1.0, 1.0).astype(np.float32)
    cosT = c32[fidx, :]
    sinT = sign[:, None] * s32[fidx, :]
    return np.ascontiguousarray(cosT), np.ascontiguousarray(sinT)


def _make_in_maps(inputs):
    x = np.asarray(inputs["x"], np.float32)
    token_positions = np.asarray(inputs["token_positions"])
    WQ = np.asarray(inputs["WQ"], np.float32)
    WK = np.asarray(inputs["WK"], np.float32)
    WV = np.asarray(inputs["WV"], np.float32)
    WO = np.asarray(inputs["WO"], np.float32)
    trimask = np.triu(np.ones((128, 128), np.float32)).astype(_BF16)

    in_maps = []
    for c in range(NCORES):
        b, hg = c // 2, c % 2
        sl = slice(hg * E, (hg + 1) * E)
        cosT, sinT = _host_tables(token_positions[b])
        in_maps.append(
            {
                "xT": np.ascontiguousarray(x[b].T).astype(_BF16),
                "wqT": np.ascontiguousarray(WQ[sl, :].T).astype(_BF16),
                "wkT": np.ascontiguousarray(WK[sl, :].T).astype(_BF16),
                "wvT": np.ascontiguousarray(WV[sl, :].T).astype(_BF16),
                "woT": np.ascontiguousarray(WO[:, sl].T).astype(_BF16),
                "cosT": cosT,
                "sinT": sinT,
                "trim": trimask,
            }
        )
    return in_maps


def kernel(x, token_positions, WQ, WK, WV, WO):
    from concourse.bass_utils import run_bass_kernel_spmd

    nc = _get_nc()
    in_maps = _make_in_maps(
        {
            "x": x,
            "token_positions": token_positions,
            "WQ": WQ,
            "WK": WK,
            "WV": WV,
            "WO": WO,
        }
    )
    res = run_bass_kernel_spmd(nc, in_maps, core_ids=list(range(NCORES)))
    out = np.empty((B, T, D), np.float32)
    for b in range(B):
        out[b] = res.results[2 * b]["y"] + res.results[2 * b + 1]["y"]
    return out
